# revision 21
# baseline (speedup 1.0000x reference)
"""Causal self-attention (B=2, S=2048, D=1024, H=16, hd=64) on 8 TRN2 cores.

Sharding: core c = (b, hg) with b = c // 4, hg = c % 4. Each core computes
attention for heads [hg*4, hg*4+4) of batch b plus its partial output
projection; the host sums the 4 partials per batch.

v2 (all-bf16, PE-tuned):
  - inputs x/W/cos/sin shipped bf16 (halved DMA), DMA interleaved with the
    QKV accumulation chains so the PE starts ~4us in.
  - scores matmuls K=64 emitted si-outer/hh-inner so consecutive matmuls
    alternate PE array rows (0/64) -- 375ns vs 625ns per 512-col matmul.
  - causal masking via narrowed diagonal matmuls + one [128,128] bf16
    lower-triangle multiply on DVE (GPSIMD affine_select eliminated).
  - softmax denominators via ones-augmented V (M=65 PV); division:
    reciprocal_approx_fast (DVE) -> partition_broadcast (GPSIMD) -> mul.
  - output projection from bf16 ot tiles, 2-chunk accumulation chains.
"""
import numpy as np

N_CORES = 8
B, S, D, H, HD = 2, 2048, 1024, 16, 64
HPC = H // 4            # heads per core = 4
HS = HPC * HD           # head-dim slice per core = 256
NKC = D // 128          # K chunks for projections = 8
NST = S // 128          # S subtiles of 128 = 16

_cached = {}


def _build_nc(reps=1):
    import concourse.bacc as bacc
    import concourse.mybir as mybir
    from concourse.tile import TileContext

    F32 = mybir.dt.float32
    BF16 = mybir.dt.bfloat16
    Exp = mybir.ActivationFunctionType.Exp
    Ln = mybir.ActivationFunctionType.Ln

    nc = bacc.Bacc()
    xT = nc.declare_dram_parameter("xT", [D, S], BF16, isOutput=False)
    wqkv = nc.declare_dram_parameter("wqkv", [D, 3 * HS], BF16, isOutput=False)
    wo_t = nc.declare_dram_parameter("wo_t", [HS, D], BF16, isOutput=False)
    cosE = nc.declare_dram_parameter("cosE", [128, S], BF16, isOutput=False)
    sinE = nc.declare_dram_parameter("sinE", [128, S], BF16, isOutput=False)
    pswap = nc.declare_dram_parameter("pswap", [128, 128], BF16, isOutput=False)
    trit = nc.declare_dram_parameter("trit", [128, 128], BF16, isOutput=False)
    tick = nc.declare_dram_parameter("tick", [1, 1], F32, isOutput=False)
    out_p = nc.declare_dram_parameter("out_p", [S, D], F32, isOutput=True)
    tock = nc.declare_dram_parameter("tock", [1, 1], F32, isOutput=True)

    with TileContext(nc) as tc:
        nc.sync.dma_start(out=tock[:], in_=tick[:])
        for _rep in range(reps):
          with tc.tile_pool(name="big", bufs=1) as bpool:
            # persistent SBUF: inputs, q/k (bf16), v (bf16), attn out (bf16)
            xt = []
            wt = []
            for k in range(NKC):
                xt.append(bpool.tile([128, S], BF16, name=f"xt{k}"))
                wt.append(bpool.tile([128, 3 * HS], BF16, name=f"w{k}"))
                nc.sync.dma_start(out=wt[k][:],
                                  in_=wqkv[k * 128:(k + 1) * 128, :])
            for k in range(NKC):
                for qq in range(4):  # split across DMA queues
                    psl = slice(qq * 32, (qq + 1) * 32)
                    nc.sync.dma_start(
                        out=xt[k][psl, :],
                        in_=xT[k * 128 + qq * 32:k * 128 + (qq + 1) * 32, :])
            cos_t = bpool.tile([128, S], BF16, name="cos_t")
            sin_t = bpool.tile([128, S], BF16, name="sin_t")
            psw_t = bpool.tile([128, 128], BF16, name="psw_t")
            tri_t = bpool.tile([128, 128], BF16, name="tri_t")
            nc.sync.dma_start(out=cos_t[:], in_=cosE[:])
            nc.sync.dma_start(out=sin_t[:], in_=sinE[:])
            nc.sync.dma_start(out=psw_t[:], in_=pswap[:])
            nc.sync.dma_start(out=tri_t[:], in_=trit[:])

            qk = {}
            for wname in ("q", "k"):
                for m in range(2):
                    qk[(wname, m)] = bpool.tile([128, S], BF16,
                                                name=f"{wname}t{m}")
            v_sb = [bpool.tile([128, HPC, HD + 1], BF16, name=f"v{st}")
                    for st in range(NST)]
            ot = [bpool.tile([128, S], BF16, name=f"ot{m}") for m in range(2)]
            num_sb = {}
            den_sb = {}
            rec_sb = {}
            for hp in range(2):
                for hh in range(2):
                    for sq in range(4):
                        num_sb[(hp, hh, sq)] = bpool.tile(
                            [64, 512], BF16, name=f"num{hp}{hh}{sq}")
                        den_sb[(hp, hh, sq)] = bpool.tile(
                            [1, 512], BF16, name=f"den{hp}{hh}{sq}")
                        rec_sb[(hp, hh, sq)] = bpool.tile(
                            [1, 512], BF16, name=f"rec{hp}{hh}{sq}")
            wo_tiles = []
            for m in range(2):
                t = bpool.tile([128, D], BF16, name=f"wo{m}")
                nc.sync.dma_start(out=t[:], in_=wo_t[m * 128:(m + 1) * 128, :])
                wo_tiles.append(t)

            with tc.tile_pool(name="work", bufs=3) as wpool, \
                 tc.tile_pool(name="ptp", bufs=2) as ppool, \
                 tc.tile_pool(name="dvp", bufs=3) as dpool:

                def qk_group(wname, m, ps_qk, ps_sw):
                    """QKV projection chain + RoPE for one 128-dim chunk."""
                    woff = (0 if wname == "q" else HS) + m * 128
                    for s in range(4):
                        sl = slice(s * 512, (s + 1) * 512)
                        acc = ps_qk.tile([128, 512], F32, tag="qkacc")
                        for kc in range(NKC):
                            nc.tensor.matmul(
                                acc[:], wt[kc][:, woff:woff + 128],
                                xt[kc][:, sl],
                                start=(kc == 0), stop=(kc == NKC - 1))
                        m1 = wpool.tile([128, 512], BF16, tag="rope_m1")
                        nc.vector.tensor_mul(m1[:], acc[:], sin_t[:, sl])
                        sw = ps_sw.tile([128, 512], F32, tag="swap")
                        nc.tensor.matmul(sw[:], psw_t[:], m1[:],
                                         start=True, stop=True)
                        m2 = wpool.tile([128, 512], F32, tag="rope_m2")
                        nc.vector.tensor_mul(m2[:], acc[:], cos_t[:, sl])
                        nc.vector.tensor_add(qk[(wname, m)][:, sl],
                                             m2[:], sw[:])

                def v_group(ps_qk):
                    for st in range(NST):
                        vt = v_sb[st]
                        nc.gpsimd.memset(vt[:, :, HD:HD + 1], 1.0)
                        acc = ps_qk.tile([128, HS], F32, tag="vacc")
                        for kc in range(NKC):
                            nc.tensor.matmul(
                                acc[:], xt[kc][:, st * 128:(st + 1) * 128],
                                wt[kc][:, 2 * HS:3 * HS],
                                start=(kc == 0), stop=(kc == NKC - 1))
                        nc.vector.tensor_copy(
                            vt[:, :, 0:HD],
                            acc[:].rearrange("p (h d) -> p h d", h=HPC))

                def attn(hp, ps_sc, ps_pv):
                    """Flash attention for head pair hp (heads 2hp, 2hp+1)."""
                    kt = qk[("k", hp)]
                    qt = qk[("q", hp)]
                    for sqh in range(2):
                        sqs = [2 * sqh, 2 * sqh + 1]
                        pv = {}
                        for hh in range(2):
                            for sq in sqs:
                                pv[(hh, sq)] = ps_pv.tile(
                                    [HD + 1, 512], F32,
                                    name=f"pv{hh}{sq % 2}",
                                    tag=f"pv{hh}{sq % 2}")
                        jmax = sqs[-1] * 4 + 3
                        for j in range(jmax + 1):
                            valid = [sq for sq in sqs if sq >= j // 4]
                            jsl = slice(j * 128, (j + 1) * 128)
                            # scores: si outer / hh inner => PE rows alternate
                            sc_t = {}
                            for si, sq in enumerate(valid):
                                o = (j - 4 * sq) * 128 if j // 4 == sq else 0
                                for hh in range(2):
                                    hsl = slice(hh * 64, hh * 64 + 64)
                                    sc = ps_sc.tile([128, 512], F32,
                                                    name=f"sc{hh}{si}",
                                                    tag=f"sc{hh}{si}")
                                    sc_t[(hh, si)] = sc
                                    nc.tensor.matmul(
                                        sc[:, o:512], kt[hsl, jsl],
                                        qt[hsl, sq * 512 + o:(sq + 1) * 512],
                                        start=True, stop=True)
                            # exp (+ causal mask on diagonal chunk)
                            pt_t = {}
                            for si, sq in enumerate(valid):
                                diag = (j // 4 == sq)
                                o = (j - 4 * sq) * 128 if diag else 0
                                for hh in range(2):
                                    sc = sc_t[(hh, si)]
                                    pt = ppool.tile([128, 512], BF16,
                                                    name=f"p{hh}{si}",
                                                    tag=f"p{hh}{si}")
                                    pt_t[(hh, si)] = pt
                                    if o > 0:
                                        nc.gpsimd.memset(pt[:, 0:o], 0.0)
                                    nc.scalar.activation(pt[:, o:512],
                                                         sc[:, o:512],
                                                         Exp, scale=0.125)
                                    if diag:
                                        nc.vector.tensor_mul(
                                            pt[:, o:o + 128],
                                            pt[:, o:o + 128], tri_t[:])
                            # PV accumulate
                            for si, sq in enumerate(valid):
                                for hh in range(2):
                                    nc.tensor.matmul(
                                        pv[(hh, sq)][:],
                                        v_sb[j][:, hp * 2 + hh, :],
                                        pt_t[(hh, si)][:],
                                        start=(j == 0),
                                        stop=(j == sq * 4 + 3))
                        # stage numerators + denominators to SBUF;
                        # reciprocal/divide deferred to phase 3
                        for hh in range(2):
                            for sq in sqs:
                                nc.vector.tensor_copy(
                                    num_sb[(hp, hh, sq)][:],
                                    pv[(hh, sq)][0:HD, :])
                                nc.vector.tensor_copy(
                                    den_sb[(hp, hh, sq)][:],
                                    pv[(hh, sq)][HD:HD + 1, :])

                # phase 1: QKV projections + RoPE
                with tc.tile_pool(name="ps_qk", bufs=2, space="PSUM") as ps_qk, \
                     tc.tile_pool(name="ps_sw", bufs=2, space="PSUM") as ps_sw:
                    qk_group("q", 0, ps_qk, ps_sw)
                    qk_group("k", 0, ps_qk, ps_sw)
                    qk_group("q", 1, ps_qk, ps_sw)
                    qk_group("k", 1, ps_qk, ps_sw)
                    v_group(ps_qk)

                # phase 2: attention
                with tc.tile_pool(name="ps_sc", bufs=1, space="PSUM") as ps_sc, \
                     tc.tile_pool(name="ps_pv", bufs=1, space="PSUM") as ps_pv:
                    attn(0, ps_sc, ps_pv)
                    attn(1, ps_sc, ps_pv)

                # phase 3: deferred division, then output projection
                def act_recip(out_ap, in_ap):
                    se = nc.scalar
                    ins = [se.lower_ap(in_ap)] + [
                        mybir.ImmediateValue(dtype=F32, value=v)
                        for v in (0.0, 1.0, 0.0)]
                    se.add_instruction(mybir.InstActivation(
                        name=se.bass.get_next_instruction_name(),
                        func=mybir.ActivationFunctionType.Reciprocal,
                        ins=ins, outs=[se.lower_ap(out_ap)]))

                with tc.tile_pool(name="ps_o", bufs=4, space="PSUM") as ps_o, \
                     tc.tile_pool(name="ostage", bufs=3) as ospool:
                    keys = [(hp, hh, sq) for hp in range(2)
                            for hh in range(2) for sq in range(4)]
                    for key in keys:
                        act_recip(rec_sb[key][:], den_sb[key][:])
                    for hp, hh, sq in keys:
                        bc = dpool.tile([64, 512], BF16, tag="bc")
                        nc.gpsimd.partition_broadcast(
                            bc[:], rec_sb[(hp, hh, sq)][:])
                        nc.vector.tensor_mul(
                            ot[hp][hh * 64:(hh + 1) * 64,
                                   sq * 512:(sq + 1) * 512],
                            num_sb[(hp, hh, sq)][:], bc[:])
                    for st in range(NST):
                        stage = ospool.tile([128, D], F32, tag="ostage")
                        for nh in range(2):
                            acc = ps_o.tile([128, 512], F32, tag="oacc")
                            for m in range(2):
                                nc.tensor.matmul(
                                    acc[:], ot[m][:, st * 128:(st + 1) * 128],
                                    wo_tiles[m][:, nh * 512:(nh + 1) * 512],
                                    start=(m == 0), stop=(m == 1))
                            nc.vector.tensor_copy(
                                stage[:, nh * 512:(nh + 1) * 512], acc[:])
                        nc.sync.dma_start(
                            out=out_p[st * 128:(st + 1) * 128, :],
                            in_=stage[:])

    nc.compile()
    return nc


def _prep_core_inputs(x, freqs_cos, freqs_sin, Wq, Wk, Wv, Wo, core):
    import ml_dtypes
    bf16 = ml_dtypes.bfloat16

    b, hg = core // 4, core % 4
    hsl = slice(hg * HS, (hg + 1) * HS)
    perm = np.concatenate([np.arange(0, HD, 2), np.arange(1, HD, 2)])

    def permute_heads(w):     # w: [HS, D] -> rope-permuted rows
        return w.reshape(HPC, HD, D)[:, perm, :].reshape(HS, D)

    cosT = freqs_cos.T                      # [32, S]
    sinT = freqs_sin.T
    cosE = np.tile(cosT, (4, 1))            # [128, S]
    sinE = np.concatenate([sinT, -sinT, sinT, -sinT], axis=0)  # pre-swap sign
    swap = (np.arange(128) // 64) * 64 + ((np.arange(128) % 64 + 32) % 64)
    pswap = np.zeros((128, 128), dtype=np.float32)
    pswap[np.arange(128), swap] = 1.0
    tri = (np.arange(128)[:, None] <= np.arange(128)[None, :])

    wqkv = np.concatenate([
        permute_heads(Wq[hsl]).T,           # [D, 256]
        permute_heads(Wk[hsl]).T,
        Wv[hsl].T,
    ], axis=1)                              # [D, 768]

    return {
        "xT": np.ascontiguousarray(x[b].T).astype(bf16),
        "wqkv": np.ascontiguousarray(wqkv).astype(bf16),
        "wo_t": np.ascontiguousarray(Wo[:, hsl].T).astype(bf16),
        "cosE": np.ascontiguousarray(cosE).astype(bf16),
        "sinE": np.ascontiguousarray(sinE).astype(bf16),
        "pswap": pswap.astype(bf16),
        "trit": tri.astype(bf16),
        "tick": np.zeros((1, 1), np.float32),
    }


def kernel(x, freqs_cos, freqs_sin, Wq, Wk, Wv, Wo):
    from concourse.bass_utils import run_bass_kernel_spmd

    x = np.asarray(x, np.float32)
    freqs_cos = np.asarray(freqs_cos, np.float32)
    freqs_sin = np.asarray(freqs_sin, np.float32)
    Wq, Wk, Wv, Wo = (np.asarray(w, np.float32) for w in (Wq, Wk, Wv, Wo))

    if "nc" not in _cached:
        _cached["nc"] = _build_nc()
    nc = _cached["nc"]

    in_maps = [
        _prep_core_inputs(x, freqs_cos, freqs_sin, Wq, Wk, Wv, Wo, c)
        for c in range(N_CORES)
    ]
    res = run_bass_kernel_spmd(nc, in_maps, list(range(N_CORES)))
    out = np.zeros((B, S, D), np.float32)
    for c in range(N_CORES):
        out[c // 4] += res.results[c]["out_p"]
    return out


# revision 27
# speedup vs baseline: 1.0366x; 1.0366x over previous
"""Causal self-attention (B=2, S=2048, D=1024, H=16, hd=64) on 8 TRN2 cores.

Sharding: core c = (b, hg) with b = c // 4, hg = c % 4. Each core computes
attention for heads [hg*4, hg*4+4) of batch b plus its partial output
projection; the host sums the 4 partials per batch.

v2 (all-bf16, PE-tuned):
  - inputs x/W/cos/sin shipped bf16 (halved DMA), DMA interleaved with the
    QKV accumulation chains so the PE starts ~4us in.
  - scores matmuls K=64 emitted si-outer/hh-inner so consecutive matmuls
    alternate PE array rows (0/64) -- 375ns vs 625ns per 512-col matmul.
  - causal masking via narrowed diagonal matmuls + one [128,128] bf16
    lower-triangle multiply on DVE (GPSIMD affine_select eliminated).
  - softmax denominators via ones-augmented V (M=65 PV); division:
    reciprocal_approx_fast (DVE) -> partition_broadcast (GPSIMD) -> mul.
  - output projection from bf16 ot tiles, 2-chunk accumulation chains.
"""
import numpy as np

N_CORES = 8
B, S, D, H, HD = 2, 2048, 1024, 16, 64
HPC = H // 4            # heads per core = 4
HS = HPC * HD           # head-dim slice per core = 256
NKC = D // 128          # K chunks for projections = 8
NST = S // 128          # S subtiles of 128 = 16

_cached = {}


def _build_nc(reps=1):
    import concourse.bacc as bacc
    import concourse.mybir as mybir
    from concourse.tile import TileContext

    F32 = mybir.dt.float32
    BF16 = mybir.dt.bfloat16
    Exp = mybir.ActivationFunctionType.Exp
    Ln = mybir.ActivationFunctionType.Ln

    nc = bacc.Bacc()
    xT = nc.declare_dram_parameter("xT", [D, S], BF16, isOutput=False)
    wqkv = nc.declare_dram_parameter("wqkv", [D, 3 * HS], BF16, isOutput=False)
    wo_t = nc.declare_dram_parameter("wo_t", [HS, D], BF16, isOutput=False)
    cosE = nc.declare_dram_parameter("cosE", [128, S], BF16, isOutput=False)
    sinE = nc.declare_dram_parameter("sinE", [128, S], BF16, isOutput=False)
    pswap = nc.declare_dram_parameter("pswap", [128, 128], BF16, isOutput=False)
    trit = nc.declare_dram_parameter("trit", [128, 128], BF16, isOutput=False)
    tick = nc.declare_dram_parameter("tick", [1, 1], F32, isOutput=False)
    out_p = nc.declare_dram_parameter("out_p", [S, D], F32, isOutput=True)
    tock = nc.declare_dram_parameter("tock", [1, 1], F32, isOutput=True)

    with TileContext(nc) as tc:
        nc.sync.dma_start(out=tock[:], in_=tick[:])
        for _rep in range(reps):
          with tc.tile_pool(name="big", bufs=1) as bpool:
            # persistent SBUF: inputs, q/k (bf16), v (bf16), attn out (bf16)
            xt = []
            wt = []
            for k in range(NKC):
                xt.append(bpool.tile([128, S], BF16, name=f"xt{k}"))
                wt.append(bpool.tile([128, 3 * HS], BF16, name=f"w{k}"))
                nc.sync.dma_start(out=xt[k][:], in_=xT[k * 128:(k + 1) * 128, :])
                nc.sync.dma_start(out=wt[k][:],
                                  in_=wqkv[k * 128:(k + 1) * 128, :])
            ones_t = bpool.tile([1, 64], BF16, name="ones_t")
            nc.vector.memset(ones_t[:], 1.0)
            cos_t = bpool.tile([128, S], BF16, name="cos_t")
            sin_t = bpool.tile([128, S], BF16, name="sin_t")
            psw_t = bpool.tile([128, 128], BF16, name="psw_t")
            tri_t = bpool.tile([128, 128], BF16, name="tri_t")
            nc.sync.dma_start(out=cos_t[:], in_=cosE[:])
            nc.sync.dma_start(out=sin_t[:], in_=sinE[:])
            nc.sync.dma_start(out=psw_t[:], in_=pswap[:])
            nc.sync.dma_start(out=tri_t[:], in_=trit[:])

            qk = {}
            for wname in ("q", "k"):
                for m in range(2):
                    qk[(wname, m)] = bpool.tile([128, S], BF16,
                                                name=f"{wname}t{m}")
            v_sb = [bpool.tile([128, HPC, HD + 1], BF16, name=f"v{st}")
                    for st in range(NST)]
            ot = [bpool.tile([128, S], BF16, name=f"ot{m}") for m in range(2)]
            num_sb = {}
            den_sb = {}
            rec_sb = {}
            for hp in range(2):
                for hh in range(2):
                    for sq in range(4):
                        num_sb[(hp, hh, sq)] = bpool.tile(
                            [64, 512], BF16, name=f"num{hp}{hh}{sq}")
                        den_sb[(hp, hh, sq)] = bpool.tile(
                            [1, 512], BF16, name=f"den{hp}{hh}{sq}")
                        rec_sb[(hp, hh, sq)] = bpool.tile(
                            [1, 512], BF16, name=f"rec{hp}{hh}{sq}")
            wo_tiles = []
            for m in range(2):
                t = bpool.tile([128, D], BF16, name=f"wo{m}")
                nc.sync.dma_start(out=t[:], in_=wo_t[m * 128:(m + 1) * 128, :])
                wo_tiles.append(t)

            with tc.tile_pool(name="work", bufs=3) as wpool, \
                 tc.tile_pool(name="ptp", bufs=2) as ppool, \
                 tc.tile_pool(name="dvp", bufs=3) as dpool:

                def qk_group(wname, m, ps_qk, ps_sw):
                    """QKV projection chain + RoPE for one 128-dim chunk."""
                    woff = (0 if wname == "q" else HS) + m * 128
                    for s in range(4):
                        sl = slice(s * 512, (s + 1) * 512)
                        acc = ps_qk.tile([128, 512], F32, tag="qkacc")
                        for kc in range(NKC):
                            nc.tensor.matmul(
                                acc[:], wt[kc][:, woff:woff + 128],
                                xt[kc][:, sl],
                                start=(kc == 0), stop=(kc == NKC - 1))
                        m1 = wpool.tile([128, 512], BF16, tag="rope_m1")
                        nc.vector.tensor_mul(m1[:], acc[:], sin_t[:, sl])
                        sw = ps_sw.tile([128, 512], F32, tag="swap")
                        nc.tensor.matmul(sw[:], psw_t[:], m1[:],
                                         start=True, stop=True)
                        m2 = wpool.tile([128, 512], F32, tag="rope_m2")
                        nc.vector.tensor_mul(m2[:], acc[:], cos_t[:, sl])
                        nc.vector.tensor_add(qk[(wname, m)][:, sl],
                                             m2[:], sw[:])

                def v_group(ps_qk):
                    for st in range(NST):
                        vt = v_sb[st]
                        nc.vector.memset(vt[:, :, HD:HD + 1], 1.0)
                        acc = ps_qk.tile([128, HS], F32, tag="vacc")
                        for kc in range(NKC):
                            nc.tensor.matmul(
                                acc[:], xt[kc][:, st * 128:(st + 1) * 128],
                                wt[kc][:, 2 * HS:3 * HS],
                                start=(kc == 0), stop=(kc == NKC - 1))
                        nc.vector.tensor_copy(
                            vt[:, :, 0:HD],
                            acc[:].rearrange("p (h d) -> p h d", h=HPC))

                def attn(hp, ps_sc, ps_pv):
                    """Flash attention for head pair hp (heads 2hp, 2hp+1)."""
                    kt = qk[("k", hp)]
                    qt = qk[("q", hp)]
                    for sqh in range(2):
                        sqs = [2 * sqh, 2 * sqh + 1]
                        pv = {}
                        for hh in range(2):
                            for sq in sqs:
                                pv[(hh, sq)] = ps_pv.tile(
                                    [HD + 1, 512], F32,
                                    name=f"pv{hh}{sq % 2}",
                                    tag=f"pv{hh}{sq % 2}")
                        jmax = sqs[-1] * 4 + 3
                        for j in range(jmax + 1):
                            valid = [sq for sq in sqs if sq >= j // 4]
                            jsl = slice(j * 128, (j + 1) * 128)
                            # scores: si outer / hh inner => PE rows alternate
                            sc_t = {}
                            for si, sq in enumerate(valid):
                                o = (j - 4 * sq) * 128 if j // 4 == sq else 0
                                for hh in range(2):
                                    hsl = slice(hh * 64, hh * 64 + 64)
                                    sc = ps_sc.tile([128, 512], F32,
                                                    name=f"sc{hh}{si}",
                                                    tag=f"sc{hh}{si}")
                                    sc_t[(hh, si)] = sc
                                    nc.tensor.matmul(
                                        sc[:, o:512], kt[hsl, jsl],
                                        qt[hsl, sq * 512 + o:(sq + 1) * 512],
                                        start=True, stop=True)
                            # exp (+ causal mask on diagonal chunk)
                            pt_t = {}
                            for si, sq in enumerate(valid):
                                diag = (j // 4 == sq)
                                o = (j - 4 * sq) * 128 if diag else 0
                                for hh in range(2):
                                    sc = sc_t[(hh, si)]
                                    pt = ppool.tile([128, 512], BF16,
                                                    name=f"p{hh}{si}",
                                                    tag=f"p{hh}{si}")
                                    pt_t[(hh, si)] = pt
                                    if o > 0:
                                        nc.vector.memset(pt[:, 0:o], 0.0)
                                    nc.scalar.activation(pt[:, o:512],
                                                         sc[:, o:512],
                                                         Exp, scale=0.125)
                                    if diag:
                                        nc.vector.tensor_mul(
                                            pt[:, o:o + 128],
                                            pt[:, o:o + 128], tri_t[:])
                            # PV accumulate
                            for si, sq in enumerate(valid):
                                for hh in range(2):
                                    nc.tensor.matmul(
                                        pv[(hh, sq)][:],
                                        v_sb[j][:, hp * 2 + hh, :],
                                        pt_t[(hh, si)][:],
                                        start=(j == 0),
                                        stop=(j == sq * 4 + 3))
                        # stage numerators + denominators to SBUF;
                        # reciprocal/divide deferred to phase 3
                        for hh in range(2):
                            for sq in sqs:
                                nc.vector.tensor_copy(
                                    num_sb[(hp, hh, sq)][:],
                                    pv[(hh, sq)][0:HD, :])
                                nc.vector.tensor_copy(
                                    den_sb[(hp, hh, sq)][:],
                                    pv[(hh, sq)][HD:HD + 1, :])

                # phase 1: QKV projections + RoPE
                with tc.tile_pool(name="ps_qk", bufs=2, space="PSUM") as ps_qk, \
                     tc.tile_pool(name="ps_sw", bufs=2, space="PSUM") as ps_sw:
                    qk_group("q", 0, ps_qk, ps_sw)
                    qk_group("k", 0, ps_qk, ps_sw)
                    qk_group("q", 1, ps_qk, ps_sw)
                    qk_group("k", 1, ps_qk, ps_sw)
                    v_group(ps_qk)

                # phase 2: attention
                with tc.tile_pool(name="ps_sc", bufs=1, space="PSUM") as ps_sc, \
                     tc.tile_pool(name="ps_pv", bufs=1, space="PSUM") as ps_pv:
                    attn(0, ps_sc, ps_pv)
                    attn(1, ps_sc, ps_pv)

                # phase 3: deferred division, then output projection
                def act_recip(out_ap, in_ap):
                    se = nc.scalar
                    ins = [se.lower_ap(in_ap)] + [
                        mybir.ImmediateValue(dtype=F32, value=v)
                        for v in (0.0, 1.0, 0.0)]
                    se.add_instruction(mybir.InstActivation(
                        name=se.bass.get_next_instruction_name(),
                        func=mybir.ActivationFunctionType.Reciprocal,
                        ins=ins, outs=[se.lower_ap(out_ap)]))

                with tc.tile_pool(name="ps_o", bufs=4, space="PSUM") as ps_o, \
                     tc.tile_pool(name="ps_bc", bufs=3, space="PSUM") as ps_bc, \
                     tc.tile_pool(name="ostage", bufs=3) as ospool:
                    keys = [(hp, hh, sq) for sq in range(4)
                            for hp in range(2) for hh in range(2)]
                    for key in keys:
                        act_recip(rec_sb[key][:], den_sb[key][:])
                    for sq in range(4):
                        # divisions for this query block (PE rank-1 bcast)
                        for hp in range(2):
                            for hh in range(2):
                                bc = ps_bc.tile([64, 512], F32, tag="bc")
                                nc.tensor.matmul(
                                    bc[:], ones_t[:],
                                    rec_sb[(hp, hh, sq)][:],
                                    start=True, stop=True)
                                nc.vector.tensor_mul(
                                    ot[hp][hh * 64:(hh + 1) * 64,
                                           sq * 512:(sq + 1) * 512],
                                    num_sb[(hp, hh, sq)][:], bc[:])
                        for st in range(4 * sq, 4 * sq + 4):
                            stage = ospool.tile([128, D], F32, tag="ostage")
                            for nh in range(2):
                                acc = ps_o.tile([128, 512], F32, tag="oacc")
                                for m in range(2):
                                    nc.tensor.matmul(
                                        acc[:],
                                        ot[m][:, st * 128:(st + 1) * 128],
                                        wo_tiles[m][:, nh * 512:(nh + 1) * 512],
                                        start=(m == 0), stop=(m == 1))
                                nc.vector.tensor_copy(
                                    stage[:, nh * 512:(nh + 1) * 512], acc[:])
                            nc.sync.dma_start(
                                out=out_p[st * 128:(st + 1) * 128, :],
                                in_=stage[:])

    nc.compile()
    return nc


def _prep_core_inputs(x, freqs_cos, freqs_sin, Wq, Wk, Wv, Wo, core):
    import ml_dtypes
    bf16 = ml_dtypes.bfloat16

    b, hg = core // 4, core % 4
    hsl = slice(hg * HS, (hg + 1) * HS)
    perm = np.concatenate([np.arange(0, HD, 2), np.arange(1, HD, 2)])

    def permute_heads(w):     # w: [HS, D] -> rope-permuted rows
        return w.reshape(HPC, HD, D)[:, perm, :].reshape(HS, D)

    cosT = freqs_cos.T                      # [32, S]
    sinT = freqs_sin.T
    cosE = np.tile(cosT, (4, 1))            # [128, S]
    sinE = np.concatenate([sinT, -sinT, sinT, -sinT], axis=0)  # pre-swap sign
    swap = (np.arange(128) // 64) * 64 + ((np.arange(128) % 64 + 32) % 64)
    pswap = np.zeros((128, 128), dtype=np.float32)
    pswap[np.arange(128), swap] = 1.0
    tri = (np.arange(128)[:, None] <= np.arange(128)[None, :])

    wqkv = np.concatenate([
        permute_heads(Wq[hsl]).T,           # [D, 256]
        permute_heads(Wk[hsl]).T,
        Wv[hsl].T,
    ], axis=1)                              # [D, 768]

    return {
        "xT": np.ascontiguousarray(x[b].T).astype(bf16),
        "wqkv": np.ascontiguousarray(wqkv).astype(bf16),
        "wo_t": np.ascontiguousarray(Wo[:, hsl].T).astype(bf16),
        "cosE": np.ascontiguousarray(cosE).astype(bf16),
        "sinE": np.ascontiguousarray(sinE).astype(bf16),
        "pswap": pswap.astype(bf16),
        "trit": tri.astype(bf16),
        "tick": np.zeros((1, 1), np.float32),
    }


def kernel(x, freqs_cos, freqs_sin, Wq, Wk, Wv, Wo):
    from concourse.bass_utils import run_bass_kernel_spmd

    x = np.asarray(x, np.float32)
    freqs_cos = np.asarray(freqs_cos, np.float32)
    freqs_sin = np.asarray(freqs_sin, np.float32)
    Wq, Wk, Wv, Wo = (np.asarray(w, np.float32) for w in (Wq, Wk, Wv, Wo))

    if "nc" not in _cached:
        _cached["nc"] = _build_nc()
    nc = _cached["nc"]

    in_maps = [
        _prep_core_inputs(x, freqs_cos, freqs_sin, Wq, Wk, Wv, Wo, c)
        for c in range(N_CORES)
    ]
    res = run_bass_kernel_spmd(nc, in_maps, list(range(N_CORES)))
    out = np.zeros((B, S, D), np.float32)
    for c in range(N_CORES):
        out[c // 4] += res.results[c]["out_p"]
    return out
